# revision 13
# baseline (speedup 1.0000x reference)
"""Trainium2 Bass kernel for nn_CustomBSplineLayer.

Math: out[b,o] = sum_{i,g} coeff[o,i,g] * w[o,i] * s_g(clip(x[b,i], -1, 1))
where s_g is a cubic B-spline basis on uniform knots (t = 3.5*(x+1) in [0,7],
8 basis functions; s_7 == 0 on the clipped domain).

Basis: each B_g is a 5-term combination of truncated powers V_q = relu(t-q)^3,
so out = sum_{q,i} P_q[b,i] * H[(q,i), o] for ANY invertible plane basis P
spanning {V_0..V_6} (H solved exactly on host, f64).  This kernel uses the
LOCAL (4th-difference / B-spline) planes

    P_q(t) = 4 * 6*Bspline(t-q) = relu(2-s)^3*4 - 4*... with s = |t-(q+2)|:
    P_q = cube(relu(2c - s')) + cube(min(s' - c, 0)),  s' = c*|t-(q+2)|,
    c = 4^(1/3)  (the input pre-scale folds the -4 coefficient away).

P_q is bounded by 16 with 4-wide support, so the plane/H product-to-output
amplification is ~0.9 (vs 27.5 for first-difference planes).  That lets BOTH
matmul operands be fp16: quantization (2^-11) gives ~2.4e-4 output error
(simulated exactly on the real inputs; fp16 x fp16 products are EXACT in the
fp32 PSUM accumulate).  fp16 weights enable FWL (fast weight load), which
f32r blocks (fp32_mode=HIGH), trimming the ~64-cycle/matmul LDWEIGHTS bubble.

Per-plane production pipeline (three engines, ~1.2us/plane each, vs 1.73us
PE consumption):
    ScalarE : s' = Abs(c*tp + c*(1.5-q))       [bias from qb tile, scale=c]
    GpSimd  : mt = min(s' - c, 0)              [tensor_scalar add/min]
    DVE     : plane = B4PAIR(s', mt) -> fp16   [7-stage custom op]

Layout (data-parallel over batch, 8 cores x 1024 rows):
  - xt = host tp = min(3.5x, 3.5), pre-transposed: [512 i, 1024 b] per core.
  - planes per (i-block, q): [128, 1024] fp16; matmul lhsT slices [128,128];
    rhs H tiles [128, 512 o] fp16; PSUM [128 b, 512 o] x 8 banks, 28 k-tiles.
  - startup: xs0 quartered across 4 DMA queues, h2[0] on gpsimd queue, so the
    first real matmul lands ~9.6us; ~10 fp16 dummies pre-warm the HAM clock
    gate from ~7.3us.  kt0 matmuls for banks 4-7 are deferred behind kt1 of
    banks 0-3 to relax the xs0 tail-quarter deadline.
  - last i-block bank-major so PSUM drains + out-DMA overlap the tail.
"""

import numpy as np

import concourse.mybir as mybir
from concourse import bacc
import concourse.tile as tile
from concourse.bass_utils import run_bass_kernel_spmd
from concourse import dve_ops as _dops
from concourse.dve_spec import Spec, Src0, Src1, C0, C1, Zero, maxx, sq
from concourse.dve_spec import lower as _dve_lower
from concourse.dve_uop import DveOpSpec as _DveOpSpec

F32 = mybir.dt.float32
F16 = mybir.dt.float16
ACTF = mybir.ActivationFunctionType
AOT = mybir.AluOpType

N_CORES = 8
BATCH, I, O, G = 8192, 512, 512, 8
BC = BATCH // N_CORES          # 1024 batch rows per core
Q = 7                          # planes q = 0..6
IB = I // 128                  # 4 i-blocks
KT = Q * IB                    # 28 k-tiles of 128
NBB = BC // 128                # 8 batch blocks of 128
CBRT4 = float(4.0 ** (1.0 / 3.0))
N_DUMMIES = 11


def _register_b4pair():
    """out = relu(s0 - in0)^3 + s1 * in1^3.

    With in0 = s' = c*|t-(q+2)|, in1 = min(s' - c, 0), s0 = 2c, s1 = 4
    (c = 4^(1/3)): out = 4*(2-s)+^3 - 16*(1-s)+^3 = 4 * [6*Bspline](t-q)."""
    name = "B4PAIR_ANT"
    for op in _dops.OPS:
        if op.name == name:
            return op

    def _ref(in0, in1, s0, s1, imm2):
        mu = np.maximum(np.float32(s0) - in0.astype(np.float32), np.float32(0))
        v = in1.astype(np.float32)
        return (mu * mu * mu + v * v * v * np.float32(s1)).astype(np.float32)

    mu = maxx(C0 - Src0, Zero)
    spec = Spec(body=sq(mu) * mu + sq(Src1) * Src1 * C1, reference=_ref)
    opcode = _dops._CUSTOM_DVE_ROW_BASE + len(_dops.OPS)
    assert opcode < 0x20
    shas = {}
    for ver in ("v3", "v4"):
        try:
            shas[ver] = _DveOpSpec(
                name=name, opcode=opcode, uops=_dve_lower(spec, ver=ver),
                rd1_en=True).sha(ver)
        except Exception:
            pass
    op = _dops.DveOp(name, spec, subdim=False, uops_sha=shas)
    _dops.OPS.append(op)
    _dops.CUSTOM_DVE_SPECS[name] = spec
    _dops._SUB_OPCODE_FOR_NAME[name] = opcode
    return op


B4PAIR = _register_b4pair()

_programs = {}


def _build_program():
    nc = bacc.Bacc("TRN2", target_bir_lowering=False, debug=False,
                   num_devices=N_CORES)
    xt_d = nc.dram_tensor("xt", [I, BC], F32, kind="ExternalInput").ap()
    h2_d = nc.dram_tensor("h2", [KT * 128, O], F16, kind="ExternalInput").ap()
    qb_d = nc.dram_tensor("qb", [128, 8], F32, kind="ExternalInput").ap()
    out_d = nc.dram_tensor("out", [BC, O], F32, kind="ExternalOutput").ap()

    with tile.TileContext(nc) as tc:
        with tc.tile_pool(name="g", bufs=1) as gpool, \
             tc.tile_pool(name="x", bufs=4) as xpool, \
             tc.tile_pool(name="a", bufs=4) as apool, \
             tc.tile_pool(name="m", bufs=4) as mpool, \
             tc.tile_pool(name="p", bufs=9) as ppool, \
             tc.tile_pool(name="o", bufs=4) as opool, \
             tc.tile_pool(name="ps", bufs=1, space="PSUM") as pspool:

            # warm tile for PE HAM warm-up dummies (fp16 like the real stream
            # so FWL state matches): memset fp32, cast to fp16.
            warm0 = gpool.tile([128, 256], F32)
            nc.gpsimd.memset(warm0[:], 0.0)
            warm = gpool.tile([128, 256], F16)
            nc.vector.tensor_copy(out=warm[:], in_=warm0[:])
            scr = gpool.tile([128, 8], F32)
            qb_s = gpool.tile([128, 8], F32)
            xs = [xpool.tile([128, BC], F32, name=f"xs{ib}", tag=f"xs{ib}")
                  for ib in range(IB)]
            h2_s = gpool.tile([128, KT, O], F16)

            # --- startup DMAs (only scalar/sync/gpsimd queues can DMA):
            # xs0 quartered across the three queues; h2[0] first on gpsimd.
            def h2_dma(eng, k0, k1):
                eng.dma_start(
                    out=h2_s[:, k0:k1, :],
                    in_=h2_d[k0 * 128:k1 * 128, :].rearrange(
                        "(kt p) o -> p kt o", p=128))

            nc.scalar.dma_start(out=xs[0][:, 0:128], in_=xt_d[0:128, 0:128])
            nc.sync.dma_start(out=qb_s[:], in_=qb_d[:])
            nc.sync.dma_start(out=xs[0][:, 128:256],
                              in_=xt_d[0:128, 128:256])
            h2_dma(nc.gpsimd, 0, 1)
            nc.scalar.dma_start(out=xs[0][:, 256:512],
                                in_=xt_d[0:128, 256:512])
            nc.gpsimd.dma_start(out=xs[0][:, 512:1024],
                                in_=xt_d[0:128, 512:1024])

            # ACT_TABLE_LOAD hoist (Abs table) while DMAs fly.
            nc.scalar.activation(scr[:], warm0[:, 0:8], ACTF.Abs, scale=1.0)

            psums = [pspool.tile([128, O], F32, name=f"ps{bb}", tag=f"ps{bb}")
                     for bb in range(NBB)]

            # PE HAM warm-up: open the clock gate (~3.4us of PE activity)
            # before the real stream starts at ~9.6us.
            for _ in range(N_DUMMIES):
                nc.tensor.matmul(psums[0][:, 0:256], warm[:, 0:128],
                                 warm[:, 0:256], start=True, stop=True)

            # h2 finely chunked across sync+gpsimd to match per-kt deadlines
            # (~9.6us + 1.73us*kt), then coarse; xs2 interleaved on sync.
            h2_dma(nc.sync, 1, 2)
            h2_dma(nc.sync, 2, 3)
            h2_dma(nc.sync, 4, 5)
            h2_dma(nc.sync, 7, 9)
            h2_dma(nc.sync, 12, 15)
            nc.sync.dma_start(out=xs[2][:], in_=xt_d[256:384, :])
            h2_dma(nc.sync, 19, 23)
            h2_dma(nc.gpsimd, 3, 4)
            h2_dma(nc.gpsimd, 5, 7)
            h2_dma(nc.gpsimd, 9, 12)
            h2_dma(nc.gpsimd, 15, 19)
            h2_dma(nc.gpsimd, 23, 28)

            def plane_ops(ib, q, c0, c1, tag, bufs):
                """ScalarE Abs (fp16) -> DVE min (4x) -> DVE B4PAIR -> fp16.

                All three tiles fp16: the tensor_scalar hits 4x perf mode
                (16-bit, step 1, single-src) so VectorE spends ~1.55us/plane
                vs the PE's 1.73us/plane consumption.  GpSimd must NOT touch
                these streams: concurrent gpsimd elementwise work collapses
                DVE throughput ~10x (SBUF-path contention, measured)."""
                w = c1 - c0
                a = apool.tile([128, w], F16, tag=f"a{tag}", bufs=bufs)
                nc.scalar.activation(a[:], xs[ib][:, c0:c1], ACTF.Abs,
                                     bias=qb_s[:, q:q + 1], scale=CBRT4)
                mt = mpool.tile([128, w], F16, tag=f"m{tag}", bufs=bufs)
                nc.vector.tensor_scalar(out=mt[:], in0=a[:],
                                        scalar1=-CBRT4, scalar2=0.0,
                                        op0=AOT.add, op1=AOT.min)
                p = ppool.tile([128, w], F16, tag=f"p{tag}", bufs=bufs)
                nc.vector._custom_dve(B4PAIR, out=p[:], in0=a[:], in1=mt[:],
                                      s0=2.0 * CBRT4, s1=4.0)
                return p

            def mm(p, pc0, kt, bb, start, stop):
                """One matmul: plane cols [bb*128 .. ) within tile offset pc0."""
                lo = bb * 128 - pc0
                nc.tensor.matmul(psums[bb][:], p[:, lo:lo + 128],
                                 h2_s[:, kt, :], start=start, stop=stop)

            # --- i-block 0, q0/q1 sliced to match DMA arrival order: banks
            # 0-3 run kt0/kt1 before banks 4-7's kt0, relaxing the xs0 tail
            # deadline.  Slice width tracks the xs0 chunk sizes above.
            pA = plane_ops(0, 0, 0, 128, "s8", 2)       # q0 cols 0:128
            mm(pA, 0, 0, 0, True, False)
            pB = plane_ops(0, 0, 128, 256, "s8", 2)     # q0 cols 128:256
            mm(pB, 128, 0, 1, True, False)
            pC = plane_ops(0, 1, 0, 256, "s4", 2)       # q1 cols 0:256
            mm(pC, 0, 1, 0, False, False)
            mm(pC, 0, 1, 1, False, False)
            pD = plane_ops(0, 0, 256, 512, "s4", 2)     # q0 cols 256:512
            mm(pD, 256, 0, 2, True, False)
            mm(pD, 256, 0, 3, True, False)
            pE = plane_ops(0, 1, 256, 512, "s4", 2)     # q1 cols 256:512
            mm(pE, 256, 1, 2, False, False)
            mm(pE, 256, 1, 3, False, False)
            pF = plane_ops(0, 0, 512, 1024, "s2", 2)    # q0 cols 512:1024
            for bb in range(4, 8):
                mm(pF, 512, 0, bb, True, False)
            pG = plane_ops(0, 1, 512, 1024, "s2", 2)    # q1 cols 512:1024
            for bb in range(4, 8):
                mm(pG, 512, 1, bb, False, False)

            # mid-stream input DMAs on the scalar queue (it has slack).
            nc.scalar.dma_start(out=xs[1][:], in_=xt_d[128:256, :])
            nc.scalar.dma_start(out=xs[3][:], in_=xt_d[384:512, :])

            # --- steady stream: full-width planes, kt-major; last i-block
            # planes are kept for the bank-major drain-overlapped finale.
            planes = {}
            for ib in range(IB):
                for q in range(Q):
                    if ib == 0 and q < 2:
                        continue
                    kt = ib * Q + q
                    p = plane_ops(ib, q, 0, BC, "", None)
                    if ib < IB - 1:
                        for bb in range(NBB):
                            mm(p, 0, kt, bb, False, False)
                    else:
                        planes[q] = p

            # last i-block: bank-major so each PSUM bank finishes ~1.5us
            # apart and its drain + out-DMA overlaps the remaining matmuls.
            for bb in range(NBB):
                for q in range(Q):
                    kt = (IB - 1) * Q + q
                    lhs = planes[q][:, bb * 128:(bb + 1) * 128]
                    if bb == NBB - 1 and q == Q - 1:
                        nc.tensor.matmul(psums[bb][:, 0:256], lhs,
                                         h2_s[:, kt, 0:256],
                                         start=False, stop=True)
                        nc.tensor.matmul(psums[bb][:, 256:O], lhs,
                                         h2_s[:, kt, 256:O],
                                         start=False, stop=True)
                        continue
                    nc.tensor.matmul(psums[bb][:], lhs, h2_s[:, kt, :],
                                     start=False, stop=(q == Q - 1))
                if bb in (1, 3, 5):
                    j = bb // 2
                    o2 = opool.tile([128, 2, O], F32, tag="o", bufs=2)
                    nc.scalar.copy(o2[:, 0, :], psums[2 * j][:])
                    nc.vector.tensor_copy(out=o2[:, 1, :],
                                          in_=psums[2 * j + 1][:])
                    nc.sync.dma_start(
                        out=out_d[2 * j * 128:(2 * j + 2) * 128, :].rearrange(
                            "(k p) o -> p k o", p=128),
                        in_=o2[:])
                elif bb == 6:
                    o6 = opool.tile([128, O], F32, tag="o1", bufs=1)
                    nc.scalar.copy(o6[:, 0:256], psums[6][:, 0:256])
                    nc.scalar.dma_start(out=out_d[6 * 128:7 * 128, 0:256],
                                        in_=o6[:, 0:256])
                    nc.vector.tensor_copy(out=o6[:, 256:O],
                                          in_=psums[6][:, 256:O])
                    nc.sync.dma_start(out=out_d[6 * 128:7 * 128, 256:O],
                                      in_=o6[:, 256:O])
                elif bb == 7:
                    # last bank: quartered drain on scalar+vector, out-DMA
                    # spread over three queues so the tail ends ~1us after
                    # the final matmul.
                    o7 = opool.tile([128, O], F32, tag="o2", bufs=1)
                    nc.vector.tensor_copy(out=o7[:, 0:128],
                                          in_=psums[7][:, 0:128])
                    nc.sync.dma_start(out=out_d[7 * 128:8 * 128, 0:128],
                                      in_=o7[:, 0:128])
                    nc.scalar.copy(o7[:, 128:256], psums[7][:, 128:256])
                    nc.scalar.dma_start(out=out_d[7 * 128:8 * 128, 128:256],
                                        in_=o7[:, 128:256])
                    nc.vector.tensor_copy(out=o7[:, 256:384],
                                          in_=psums[7][:, 256:384])
                    nc.gpsimd.dma_start(out=out_d[7 * 128:8 * 128, 256:384],
                                        in_=o7[:, 256:384])
                    nc.scalar.copy(o7[:, 384:O], psums[7][:, 384:O])
                    nc.sync.dma_start(out=out_d[7 * 128:8 * 128, 384:O],
                                      in_=o7[:, 384:O])

    nc.compile()
    return nc


def _get_program():
    if "p" not in _programs:
        _programs["p"] = _build_program()
    return _programs["p"]


def _host_prep(x, weights, coefficients):
    x = np.asarray(x, dtype=np.float32)
    weights = np.asarray(weights, dtype=np.float32)
    coefficients = np.asarray(coefficients, dtype=np.float32)

    # raw truncated-power coefficients G_q = sum_g w5[q-g]/6 * C2_g
    c2 = coefficients.astype(np.float64) * weights.astype(np.float64)[:, :, None]
    c2 = c2.transpose(2, 1, 0)                     # [G, I, O]
    w5 = np.array([1.0, -4.0, 6.0, -4.0, 1.0]) / 6.0
    graw = np.zeros((Q, I, O), dtype=np.float64)
    for q in range(Q):
        for g in range(G):
            r = q - g
            if 0 <= r <= 4:
                graw[q] += w5[r] * c2[g]
    # local planes P_q = 4 * [V_q -4V_{q+1} +6V_{q+2} -4V_{q+3} +V_{q+4}]
    # (indices >= 7 vanish on the domain) => H = (M4^T)^{-1} G / 4
    w4 = np.array([1.0, -4.0, 6.0, -4.0, 1.0])
    M4 = np.eye(Q)
    for q in range(Q):
        for r in range(1, 5):
            if q + r < Q:
                M4[q, q + r] = w4[r]
    h = np.linalg.solve(M4.T, graw.reshape(Q, -1)).reshape(Q, I, O) / 4.0
    # device row order kt = ib*7 + q
    h2k = np.empty((KT, 128, O), dtype=np.float16)
    for ib in range(IB):
        for q in range(Q):
            h2k[ib * Q + q] = h[q, ib * 128:(ib + 1) * 128, :].astype(
                np.float16)
    h2k = np.ascontiguousarray(h2k.reshape(KT * 128, O))

    # tp = min(3.5*x, 3.5); planes vanish for t<0 via the |.| window.
    tp = np.minimum(3.5 * x, np.float32(3.5)).astype(np.float32)
    xt = np.ascontiguousarray(tp.T)                # [I, B]
    # s'_q = |c*tp + c*(1.5 - q)| = c*|t - (q+2)|,  t = tp + 3.5
    qb = np.tile((CBRT4 * (1.5 - np.arange(8, dtype=np.float64))
                  ).astype(np.float32)[None, :], (128, 1))

    in_maps = []
    for c in range(N_CORES):
        in_maps.append({
            "xt": np.ascontiguousarray(xt[:, c * BC:(c + 1) * BC]),
            "h2": h2k,
            "qb": qb,
        })
    return in_maps


def _run(x, weights, coefficients, **spmd_kwargs):
    nc = _get_program()
    in_maps = _host_prep(x, weights, coefficients)
    res = run_bass_kernel_spmd(nc, in_maps, list(range(N_CORES)), **spmd_kwargs)
    out = np.concatenate([res.results[c]["out"] for c in range(N_CORES)], axis=0)
    return out.astype(np.float32), res


def kernel(x, weights, coefficients):
    out, _ = _run(x, weights, coefficients)
    return out


# revision 18
# speedup vs baseline: 1.0531x; 1.0531x over previous
"""Trainium2 Bass kernel for nn_CustomBSplineLayer.

Math: out[b,o] = sum_{i,g} coeff[o,i,g] * w[o,i] * s_g(clip(x[b,i], -1, 1))
where s_g is a cubic B-spline basis on uniform knots (t = 3.5*(x+1) in [0,7],
8 basis functions; s_7 == 0 on the clipped domain).

Basis: each B_g is a 5-term combination of truncated powers V_q = relu(t-q)^3,
so out = sum_{q,i} P_q[b,i] * H[(q,i), o] for ANY invertible plane basis P
spanning {V_0..V_6} (H solved exactly on host, f64).  This kernel uses the
LOCAL (4th-difference / B-spline) planes

    P_q(t) = 4 * 6*Bspline(t-q) = relu(2-s)^3*4 - 4*... with s = |t-(q+2)|:
    P_q = cube(relu(2c - s')) + cube(min(s' - c, 0)),  s' = c*|t-(q+2)|,
    c = 4^(1/3)  (the input pre-scale folds the -4 coefficient away).

P_q is bounded by 16 with 4-wide support, so the plane/H product-to-output
amplification is ~0.9 (vs 27.5 for first-difference planes).  That lets BOTH
matmul operands be fp16: quantization (2^-11) gives ~2.4e-4 output error
(simulated exactly on the real inputs; fp16 x fp16 products are EXACT in the
fp32 PSUM accumulate).  fp16 weights enable FWL (fast weight load), which
f32r blocks (fp32_mode=HIGH), trimming the ~64-cycle/matmul LDWEIGHTS bubble.

Per-plane production pipeline (three engines, ~1.2us/plane each, vs 1.73us
PE consumption):
    ScalarE : s' = Abs(c*tp + c*(1.5-q))       [bias from qb tile, scale=c]
    GpSimd  : mt = min(s' - c, 0)              [tensor_scalar add/min]
    DVE     : plane = B4PAIR(s', mt) -> fp16   [7-stage custom op]

Layout (data-parallel over batch, 8 cores x 1024 rows):
  - xt = host tp = min(3.5x, 3.5), pre-transposed: [512 i, 1024 b] per core.
  - planes per (i-block, q): [128, 1024] fp16; matmul lhsT slices [128,128];
    rhs H tiles [128, 512 o] fp16; PSUM [128 b, 512 o] x 8 banks, 28 k-tiles.
  - startup: xs0 quartered across 4 DMA queues, h2[0] on gpsimd queue, so the
    first real matmul lands ~9.6us; ~10 fp16 dummies pre-warm the HAM clock
    gate from ~7.3us.  kt0 matmuls for banks 4-7 are deferred behind kt1 of
    banks 0-3 to relax the xs0 tail-quarter deadline.
  - last i-block bank-major so PSUM drains + out-DMA overlap the tail.
"""

import numpy as np

import concourse.mybir as mybir
from concourse import bacc
import concourse.tile as tile
from concourse.bass_utils import run_bass_kernel_spmd
from concourse import dve_ops as _dops
from concourse.dve_spec import Spec, Src0, Src1, C0, C1, Zero, maxx, sq
from concourse.dve_spec import lower as _dve_lower
from concourse.dve_uop import DveOpSpec as _DveOpSpec

F32 = mybir.dt.float32
F16 = mybir.dt.float16
ACTF = mybir.ActivationFunctionType
AOT = mybir.AluOpType

N_CORES = 8
BATCH, I, O, G = 8192, 512, 512, 8
BC = BATCH // N_CORES          # 1024 batch rows per core
Q = 7                          # planes q = 0..6
IB = I // 128                  # 4 i-blocks
KT = Q * IB                    # 28 k-tiles of 128
NBB = BC // 128                # 8 batch blocks of 128
CBRT4 = float(4.0 ** (1.0 / 3.0))
N_DUMMIES = 17


def _register_b4pair():
    """out = relu(s0 - in0)^3 + s1 * in1^3.

    With in0 = s' = c*|t-(q+2)|, in1 = min(s' - c, 0), s0 = 2c, s1 = 4
    (c = 4^(1/3)): out = 4*(2-s)+^3 - 16*(1-s)+^3 = 4 * [6*Bspline](t-q)."""
    name = "B4PAIR_ANT"
    for op in _dops.OPS:
        if op.name == name:
            return op

    def _ref(in0, in1, s0, s1, imm2):
        mu = np.maximum(np.float32(s0) - in0.astype(np.float32), np.float32(0))
        v = in1.astype(np.float32)
        return (mu * mu * mu + v * v * v * np.float32(s1)).astype(np.float32)

    mu = maxx(C0 - Src0, Zero)
    spec = Spec(body=sq(mu) * mu + sq(Src1) * Src1 * C1, reference=_ref)
    opcode = _dops._CUSTOM_DVE_ROW_BASE + len(_dops.OPS)
    assert opcode < 0x20
    shas = {}
    for ver in ("v3", "v4"):
        try:
            shas[ver] = _DveOpSpec(
                name=name, opcode=opcode, uops=_dve_lower(spec, ver=ver),
                rd1_en=True).sha(ver)
        except Exception:
            pass
    op = _dops.DveOp(name, spec, subdim=False, uops_sha=shas)
    _dops.OPS.append(op)
    _dops.CUSTOM_DVE_SPECS[name] = spec
    _dops._SUB_OPCODE_FOR_NAME[name] = opcode
    return op


B4PAIR = _register_b4pair()

_programs = {}


def _build_program():
    nc = bacc.Bacc("TRN2", target_bir_lowering=False, debug=False,
                   num_devices=N_CORES)
    xt_d = nc.dram_tensor("xt", [I, BC], F32, kind="ExternalInput").ap()
    h2_d = nc.dram_tensor("h2", [KT * 128, O], F16, kind="ExternalInput").ap()
    qb_d = nc.dram_tensor("qb", [128, 8], F32, kind="ExternalInput").ap()
    out_d = nc.dram_tensor("out", [BC, O], F32, kind="ExternalOutput").ap()

    with tile.TileContext(nc) as tc:
        with tc.tile_pool(name="g", bufs=1) as gpool, \
             tc.tile_pool(name="x", bufs=4) as xpool, \
             tc.tile_pool(name="a", bufs=4) as apool, \
             tc.tile_pool(name="m", bufs=4) as mpool, \
             tc.tile_pool(name="p", bufs=9) as ppool, \
             tc.tile_pool(name="o", bufs=4) as opool, \
             tc.tile_pool(name="ps", bufs=1, space="PSUM") as pspool:

            # warm tile for PE HAM warm-up dummies (fp16 like the real stream
            # so FWL state matches): memset fp32, cast to fp16.
            warm0 = gpool.tile([128, 256], F32)
            nc.gpsimd.memset(warm0[:], 0.0)
            warm = gpool.tile([128, 256], F16)
            nc.vector.tensor_copy(out=warm[:], in_=warm0[:])
            scr = gpool.tile([128, 8], F32)
            qb_s = gpool.tile([128, 8], F32)
            xs = [xpool.tile([128, BC], F32, name=f"xs{ib}", tag=f"xs{ib}")
                  for ib in range(IB)]
            h2_s = gpool.tile([128, KT, O], F16)

            # --- startup DMAs (only scalar/sync/gpsimd queues can DMA):
            # xs0 quartered across the three queues; h2[0] first on gpsimd.
            def h2_dma(eng, k0, k1):
                eng.dma_start(
                    out=h2_s[:, k0:k1, :],
                    in_=h2_d[k0 * 128:k1 * 128, :].rearrange(
                        "(kt p) o -> p kt o", p=128))

            nc.scalar.dma_start(out=xs[0][:, 0:128], in_=xt_d[0:128, 0:128])
            nc.sync.dma_start(out=qb_s[:], in_=qb_d[:])
            nc.sync.dma_start(out=xs[0][:, 128:256],
                              in_=xt_d[0:128, 128:256])
            h2_dma(nc.gpsimd, 0, 1)
            nc.scalar.dma_start(out=xs[0][:, 256:512],
                                in_=xt_d[0:128, 256:512])
            nc.gpsimd.dma_start(out=xs[0][:, 512:768],
                                in_=xt_d[0:128, 512:768])
            nc.scalar.dma_start(out=xs[0][:, 768:1024],
                                in_=xt_d[0:128, 768:1024])

            # ACT_TABLE_LOAD hoist (Abs table) while DMAs fly.
            nc.scalar.activation(scr[:], warm0[:, 0:8], ACTF.Abs, scale=1.0)

            psums = [pspool.tile([128, O], F32, name=f"ps{bb}", tag=f"ps{bb}")
                     for bb in range(NBB)]

            # PE HAM warm-up: open the clock gate (~3.4us of PE activity)
            # before the real stream starts at ~9.6us.
            for _ in range(N_DUMMIES):
                nc.tensor.matmul(psums[0][:, 0:256], warm[:, 0:128],
                                 warm[:, 0:256], start=True, stop=True)

            # h2 finely chunked across sync+gpsimd to match per-kt deadlines
            # (~9.6us + 1.73us*kt), then coarse; xs2 interleaved on sync.
            h2_dma(nc.sync, 1, 2)
            h2_dma(nc.sync, 2, 3)
            h2_dma(nc.sync, 4, 5)
            h2_dma(nc.sync, 7, 9)
            h2_dma(nc.sync, 12, 15)
            nc.sync.dma_start(out=xs[2][:], in_=xt_d[256:384, :])
            h2_dma(nc.sync, 19, 23)
            h2_dma(nc.gpsimd, 3, 4)
            h2_dma(nc.gpsimd, 5, 7)
            h2_dma(nc.gpsimd, 9, 12)
            h2_dma(nc.gpsimd, 15, 19)
            h2_dma(nc.gpsimd, 23, 28)

            def plane_ops(ib, q, c0, c1, tag, bufs):
                """ScalarE Abs (fp16) -> DVE min (4x) -> DVE B4PAIR -> fp16.

                All three tiles fp16: the tensor_scalar hits 4x perf mode
                (16-bit, step 1, single-src) so VectorE spends ~1.55us/plane
                vs the PE's 1.73us/plane consumption.  GpSimd must NOT touch
                these streams: concurrent gpsimd elementwise work collapses
                DVE throughput ~10x (SBUF-path contention, measured)."""
                w = c1 - c0
                a = apool.tile([128, w], F16, tag=f"a{tag}", bufs=bufs)
                nc.scalar.activation(a[:], xs[ib][:, c0:c1], ACTF.Abs,
                                     bias=qb_s[:, q:q + 1], scale=CBRT4)
                mt = mpool.tile([128, w], F16, tag=f"m{tag}", bufs=bufs)
                nc.vector.tensor_scalar(out=mt[:], in0=a[:],
                                        scalar1=-CBRT4, scalar2=0.0,
                                        op0=AOT.add, op1=AOT.min)
                p = ppool.tile([128, w], F16, tag=f"p{tag}", bufs=bufs)
                nc.vector._custom_dve(B4PAIR, out=p[:], in0=a[:], in1=mt[:],
                                      s0=2.0 * CBRT4, s1=4.0)
                return p

            def mm(p, pc0, kt, bb, start, stop):
                """One matmul: plane cols [bb*128 .. ) within tile offset pc0."""
                lo = bb * 128 - pc0
                nc.tensor.matmul(psums[bb][:], p[:, lo:lo + 128],
                                 h2_s[:, kt, :], start=start, stop=stop)

            # --- i-block 0, q0/q1 sliced to match DMA arrival order: banks
            # 0-3 run kt0/kt1 before banks 4-7's kt0, relaxing the xs0 tail
            # deadline.  Slice width tracks the xs0 chunk sizes above.
            pA = plane_ops(0, 0, 0, 128, "s8", 2)       # q0 cols 0:128
            mm(pA, 0, 0, 0, True, False)
            pB = plane_ops(0, 0, 128, 256, "s8", 2)     # q0 cols 128:256
            mm(pB, 128, 0, 1, True, False)
            pC = plane_ops(0, 1, 0, 256, "s4", 2)       # q1 cols 0:256
            mm(pC, 0, 1, 0, False, False)
            mm(pC, 0, 1, 1, False, False)
            pD = plane_ops(0, 0, 256, 512, "s4", 2)     # q0 cols 256:512
            mm(pD, 256, 0, 2, True, False)
            mm(pD, 256, 0, 3, True, False)
            pE = plane_ops(0, 1, 256, 512, "s4", 2)     # q1 cols 256:512
            mm(pE, 256, 1, 2, False, False)
            mm(pE, 256, 1, 3, False, False)
            pF1 = plane_ops(0, 0, 512, 768, "s4", 2)    # q0 cols 512:768
            mm(pF1, 512, 0, 4, True, False)
            mm(pF1, 512, 0, 5, True, False)
            pG1 = plane_ops(0, 1, 512, 768, "s4", 2)    # q1 cols 512:768
            mm(pG1, 512, 1, 4, False, False)
            mm(pG1, 512, 1, 5, False, False)
            pF2 = plane_ops(0, 0, 768, 1024, "s4", 2)   # q0 cols 768:1024
            mm(pF2, 768, 0, 6, True, False)
            mm(pF2, 768, 0, 7, True, False)
            pG2 = plane_ops(0, 1, 768, 1024, "s4", 2)   # q1 cols 768:1024
            mm(pG2, 768, 1, 6, False, False)
            mm(pG2, 768, 1, 7, False, False)

            # mid-stream input DMAs on the scalar queue (it has slack).
            nc.scalar.dma_start(out=xs[1][:], in_=xt_d[128:256, :])
            nc.scalar.dma_start(out=xs[3][:], in_=xt_d[384:512, :])

            # --- steady stream: full-width planes, kt-major; last i-block
            # planes are kept for the bank-major drain-overlapped finale.
            planes = {}
            for ib in range(IB):
                for q in range(Q):
                    if ib == 0 and q < 2:
                        continue
                    kt = ib * Q + q
                    p = plane_ops(ib, q, 0, BC, "", None)
                    if ib < IB - 1:
                        for bb in range(NBB):
                            mm(p, 0, kt, bb, False, False)
                    else:
                        planes[q] = p

            # last i-block: bank-major so each PSUM bank finishes ~1.5us
            # apart and its drain + out-DMA overlaps the remaining matmuls.
            for bb in range(NBB):
                for q in range(Q):
                    kt = (IB - 1) * Q + q
                    lhs = planes[q][:, bb * 128:(bb + 1) * 128]
                    if bb == NBB - 1 and q == Q - 1:
                        nc.tensor.matmul(psums[bb][:, 0:256], lhs,
                                         h2_s[:, kt, 0:256],
                                         start=False, stop=True)
                        nc.tensor.matmul(psums[bb][:, 256:O], lhs,
                                         h2_s[:, kt, 256:O],
                                         start=False, stop=True)
                        continue
                    nc.tensor.matmul(psums[bb][:], lhs, h2_s[:, kt, :],
                                     start=False, stop=(q == Q - 1))
                if bb in (1, 3, 5):
                    j = bb // 2
                    o2 = opool.tile([128, 2, O], F32, tag="o", bufs=2)
                    nc.scalar.copy(o2[:, 0, :], psums[2 * j][:])
                    nc.vector.tensor_copy(out=o2[:, 1, :],
                                          in_=psums[2 * j + 1][:])
                    nc.sync.dma_start(
                        out=out_d[2 * j * 128:(2 * j + 2) * 128, :].rearrange(
                            "(k p) o -> p k o", p=128),
                        in_=o2[:])
                elif bb == 6:
                    o6 = opool.tile([128, O], F32, tag="o1", bufs=1)
                    nc.scalar.copy(o6[:, 0:256], psums[6][:, 0:256])
                    nc.scalar.dma_start(out=out_d[6 * 128:7 * 128, 0:256],
                                        in_=o6[:, 0:256])
                    nc.vector.tensor_copy(out=o6[:, 256:O],
                                          in_=psums[6][:, 256:O])
                    nc.sync.dma_start(out=out_d[6 * 128:7 * 128, 256:O],
                                      in_=o6[:, 256:O])
                elif bb == 7:
                    # last bank: quartered drain on scalar+vector, out-DMA
                    # spread over three queues so the tail ends ~1us after
                    # the final matmul.
                    o7 = opool.tile([128, O], F32, tag="o2", bufs=1)
                    nc.vector.tensor_copy(out=o7[:, 0:128],
                                          in_=psums[7][:, 0:128])
                    nc.sync.dma_start(out=out_d[7 * 128:8 * 128, 0:128],
                                      in_=o7[:, 0:128])
                    nc.scalar.copy(o7[:, 128:256], psums[7][:, 128:256])
                    nc.scalar.dma_start(out=out_d[7 * 128:8 * 128, 128:256],
                                        in_=o7[:, 128:256])
                    nc.vector.tensor_copy(out=o7[:, 256:384],
                                          in_=psums[7][:, 256:384])
                    nc.gpsimd.dma_start(out=out_d[7 * 128:8 * 128, 256:384],
                                        in_=o7[:, 256:384])
                    nc.scalar.copy(o7[:, 384:O], psums[7][:, 384:O])
                    nc.sync.dma_start(out=out_d[7 * 128:8 * 128, 384:O],
                                      in_=o7[:, 384:O])

    nc.compile()
    return nc


def _get_program():
    if "p" not in _programs:
        _programs["p"] = _build_program()
    return _programs["p"]


def _host_prep(x, weights, coefficients):
    x = np.asarray(x, dtype=np.float32)
    weights = np.asarray(weights, dtype=np.float32)
    coefficients = np.asarray(coefficients, dtype=np.float32)

    # raw truncated-power coefficients G_q = sum_g w5[q-g]/6 * C2_g
    c2 = coefficients.astype(np.float64) * weights.astype(np.float64)[:, :, None]
    c2 = c2.transpose(2, 1, 0)                     # [G, I, O]
    w5 = np.array([1.0, -4.0, 6.0, -4.0, 1.0]) / 6.0
    graw = np.zeros((Q, I, O), dtype=np.float64)
    for q in range(Q):
        for g in range(G):
            r = q - g
            if 0 <= r <= 4:
                graw[q] += w5[r] * c2[g]
    # local planes P_q = 4 * [V_q -4V_{q+1} +6V_{q+2} -4V_{q+3} +V_{q+4}]
    # (indices >= 7 vanish on the domain) => H = (M4^T)^{-1} G / 4
    w4 = np.array([1.0, -4.0, 6.0, -4.0, 1.0])
    M4 = np.eye(Q)
    for q in range(Q):
        for r in range(1, 5):
            if q + r < Q:
                M4[q, q + r] = w4[r]
    h = np.linalg.solve(M4.T, graw.reshape(Q, -1)).reshape(Q, I, O) / 4.0
    # device row order kt = ib*7 + q
    h2k = np.empty((KT, 128, O), dtype=np.float16)
    for ib in range(IB):
        for q in range(Q):
            h2k[ib * Q + q] = h[q, ib * 128:(ib + 1) * 128, :].astype(
                np.float16)
    h2k = np.ascontiguousarray(h2k.reshape(KT * 128, O))

    # tp = min(3.5*x, 3.5); planes vanish for t<0 via the |.| window.
    tp = np.minimum(3.5 * x, np.float32(3.5)).astype(np.float32)
    xt = np.ascontiguousarray(tp.T)                # [I, B]
    # s'_q = |c*tp + c*(1.5 - q)| = c*|t - (q+2)|,  t = tp + 3.5
    qb = np.tile((CBRT4 * (1.5 - np.arange(8, dtype=np.float64))
                  ).astype(np.float32)[None, :], (128, 1))

    in_maps = []
    for c in range(N_CORES):
        in_maps.append({
            "xt": np.ascontiguousarray(xt[:, c * BC:(c + 1) * BC]),
            "h2": h2k,
            "qb": qb,
        })
    return in_maps


def _run(x, weights, coefficients, **spmd_kwargs):
    nc = _get_program()
    in_maps = _host_prep(x, weights, coefficients)
    res = run_bass_kernel_spmd(nc, in_maps, list(range(N_CORES)), **spmd_kwargs)
    out = np.concatenate([res.results[c]["out"] for c in range(N_CORES)], axis=0)
    return out.astype(np.float32), res


def kernel(x, weights, coefficients):
    out, _ = _run(x, weights, coefficients)
    return out


# revision 19
# speedup vs baseline: 1.0900x; 1.0350x over previous
"""Trainium2 Bass kernel for nn_CustomBSplineLayer.

Math: out[b,o] = sum_{i,g} coeff[o,i,g] * w[o,i] * s_g(clip(x[b,i], -1, 1))
where s_g is a cubic B-spline basis on uniform knots (t = 3.5*(x+1) in [0,7],
8 basis functions; s_7 == 0 on the clipped domain).

Basis: each B_g is a 5-term combination of truncated powers V_q = relu(t-q)^3,
so out = sum_{q,i} P_q[b,i] * H[(q,i), o] for ANY invertible plane basis P
spanning {V_0..V_6} (H solved exactly on host, f64).  This kernel uses the
LOCAL (4th-difference / B-spline) planes

    P_q(t) = 4 * [6*Bspline](t-q) = 4(2-s)+^3 - 16(1-s)+^3,  s = |t-(q+2)|
           = cube(relu(2c - s')) + 4*cube(min(s' - c, 0)),   s' = c*s,
    c = 4^(1/3).

P_q is bounded by 16 with 4-wide support, so the plane/H product-to-output
amplification is ~0.9 (vs 27.5 for first-difference planes).  That lets the
whole data path be fp16: tp, s', planes and H quantization (2^-11) give
~5.2e-4 output error (simulated exactly on the real inputs; fp16 x fp16
products are EXACT in the fp32 PSUM accumulate).  fp16 weights enable FWL
(fast weight load), which f32r blocks (fp32_mode=HIGH): matmul spacing drops
from ~240ns to ~216ns, and DMA falls to 6.5MB/core.

Per-plane production (ScalarE -> VectorE; GpSimd must NOT run elementwise
work - concurrent gpsimd tensor ops collapse DVE throughput ~10x, measured):
    ScalarE : s' = Abs(c*tp + c*(1.5-q)) -> fp16   [bias from qb, scale=c]
    VectorE : mt = min(s' - c, 0)        -> fp16   [tensor_scalar, 4x mode]
    VectorE : plane = B4PAIR(s', mt)     -> fp16   [8-stage custom op]
Full-width planes are produced in q-PAIRS ([128, 2, 1024] tiles) to halve
the ~0.6us/op fixed overhead: vector pace ~1.25us/plane vs the PE's 1.73,
so production runs ahead and the final bank-major sweep is pure PE speed.

Schedule (data-parallel over batch, 8 cores x 1024 rows):
  - ~12 fp16 dummy matmuls from ~7.9us open the HAM clock gate (4096-cycle
    free-running activity window; burst + dense early stream -> K=8/8 by
    ~11.2us with no mid-stream re-throttle).
  - xs0 sliced across the 3 DMA queues (scalar/sync/gpsimd) so the first
    real matmul lands ~10.3us; h2 chunks scheduled against per-kt deadlines
    (~10.3us + 1.73us*kt).
  - i-block 0 planes are column-sliced to match DMA arrival; banks 4-7's
    kt0/kt1 matmuls are deferred behind kt0/kt1 of banks 0-3 (PSUM middle
    accumulation order is free; only start/stop must bracket).
  - last i-block bank-major; PSUM drains + out-DMA overlap the tail, last
    bank quartered across scalar/vector with 3-queue out-DMA.
"""

import numpy as np

import concourse.mybir as mybir
from concourse import bacc
import concourse.tile as tile
from concourse.bass_utils import run_bass_kernel_spmd
from concourse import dve_ops as _dops
from concourse.dve_spec import Spec, Src0, Src1, C0, C1, Zero, maxx, sq
from concourse.dve_spec import lower as _dve_lower
from concourse.dve_uop import DveOpSpec as _DveOpSpec

F32 = mybir.dt.float32
F16 = mybir.dt.float16
ACTF = mybir.ActivationFunctionType
AOT = mybir.AluOpType

N_CORES = 8
BATCH, I, O, G = 8192, 512, 512, 8
BC = BATCH // N_CORES          # 1024 batch rows per core
Q = 7                          # planes q = 0..6
IB = I // 128                  # 4 i-blocks
KT = Q * IB                    # 28 k-tiles of 128
NBB = BC // 128                # 8 batch blocks of 128
CBRT4 = float(4.0 ** (1.0 / 3.0))
N_DUMMIES = 12


def _register_b4pair():
    """out = relu(s0 - in0)^3 + s1 * in1^3.

    With in0 = s' = c*|t-(q+2)|, in1 = min(s' - c, 0), s0 = 2c, s1 = 4
    (c = 4^(1/3)): out = 4(2-s)+^3 - 16(1-s)+^3 = 4 * [6*Bspline](t-q)."""
    name = "B4PAIR_ANT"
    for op in _dops.OPS:
        if op.name == name:
            return op

    def _ref(in0, in1, s0, s1, imm2):
        mu = np.maximum(np.float32(s0) - in0.astype(np.float32), np.float32(0))
        v = in1.astype(np.float32)
        return (mu * mu * mu + v * v * v * np.float32(s1)).astype(np.float32)

    mu = maxx(C0 - Src0, Zero)
    spec = Spec(body=sq(mu) * mu + sq(Src1) * Src1 * C1, reference=_ref)
    opcode = _dops._CUSTOM_DVE_ROW_BASE + len(_dops.OPS)
    assert opcode < 0x20
    shas = {}
    for ver in ("v3", "v4"):
        try:
            shas[ver] = _DveOpSpec(
                name=name, opcode=opcode, uops=_dve_lower(spec, ver=ver),
                rd1_en=True).sha(ver)
        except Exception:
            pass
    op = _dops.DveOp(name, spec, subdim=False, uops_sha=shas)
    _dops.OPS.append(op)
    _dops.CUSTOM_DVE_SPECS[name] = spec
    _dops._SUB_OPCODE_FOR_NAME[name] = opcode
    return op


B4PAIR = _register_b4pair()

_programs = {}


def _build_program():
    nc = bacc.Bacc("TRN2", target_bir_lowering=False, debug=False,
                   num_devices=N_CORES)
    xt_d = nc.dram_tensor("xt", [I, BC], F16, kind="ExternalInput").ap()
    h2_d = nc.dram_tensor("h2", [KT * 128, O], F16, kind="ExternalInput").ap()
    qb_d = nc.dram_tensor("qb", [128, 8], F32, kind="ExternalInput").ap()
    out_d = nc.dram_tensor("out", [BC, O], F32, kind="ExternalOutput").ap()

    with tile.TileContext(nc) as tc:
        with tc.tile_pool(name="g", bufs=1) as gpool, \
             tc.tile_pool(name="x", bufs=4) as xpool, \
             tc.tile_pool(name="a", bufs=4) as apool, \
             tc.tile_pool(name="m", bufs=4) as mpool, \
             tc.tile_pool(name="p", bufs=9) as ppool, \
             tc.tile_pool(name="o", bufs=4) as opool, \
             tc.tile_pool(name="ps", bufs=1, space="PSUM") as pspool:

            # warm tile for HAM warm-up dummies (fp16 like the real stream
            # so FWL state matches): memset fp32, cast to fp16.
            warm0 = gpool.tile([128, 256], F32)
            nc.gpsimd.memset(warm0[:], 0.0)
            warm = gpool.tile([128, 256], F16)
            nc.vector.tensor_copy(out=warm[:], in_=warm0[:])
            qb_s = gpool.tile([128, 8], F32)
            xs = [xpool.tile([128, BC], F16, name=f"xs{ib}", tag=f"xs{ib}")
                  for ib in range(IB)]
            h2_s = gpool.tile([128, KT, O], F16)

            def h2_dma(eng, k0, k1):
                eng.dma_start(
                    out=h2_s[:, k0:k1, :],
                    in_=h2_d[k0 * 128:k1 * 128, :].rearrange(
                        "(kt p) o -> p kt o", p=128))

            # startup DMAs, greedy by deadline across the three queues
            # (~67GB/s each; fp16 xs0 quarter = 64KB ~ 1.7us):
            nc.scalar.dma_start(out=xs[0][:, 0:256], in_=xt_d[0:128, 0:256])
            nc.sync.dma_start(out=qb_s[:], in_=qb_d[:])
            nc.sync.dma_start(out=xs[0][:, 256:512],
                              in_=xt_d[0:128, 256:512])
            h2_dma(nc.gpsimd, 0, 1)
            nc.gpsimd.dma_start(out=xs[0][:, 512:1024],
                                in_=xt_d[0:128, 512:1024])

            psums = [pspool.tile([128, O], F32, name=f"ps{bb}", tag=f"ps{bb}")
                     for bb in range(NBB)]

            # PE HAM warm-up burst; the dense real stream from ~10.4us keeps
            # the activity window busy until the gate opens.
            for _ in range(N_DUMMIES):
                nc.tensor.matmul(psums[0][:, 0:256], warm[:, 0:128],
                                 warm[:, 0:256], start=True, stop=True)

            # h2 finely chunked against per-kt deadlines, then coarse;
            # xs2 on sync mid-run.
            h2_dma(nc.sync, 1, 2)
            h2_dma(nc.sync, 2, 3)
            h2_dma(nc.sync, 4, 6)
            h2_dma(nc.sync, 8, 11)
            nc.sync.dma_start(out=xs[2][:], in_=xt_d[256:384, :])
            h2_dma(nc.sync, 19, 23)
            h2_dma(nc.gpsimd, 3, 4)
            h2_dma(nc.gpsimd, 6, 8)
            h2_dma(nc.gpsimd, 11, 12)
            h2_dma(nc.gpsimd, 15, 19)
            h2_dma(nc.gpsimd, 23, 28)

            def plane_ops(ib, q, c0, c1, tag, bufs):
                """Single-plane production (used for slices / ib0)."""
                w = c1 - c0
                a = apool.tile([128, w], F16, tag=f"a{tag}", bufs=bufs)
                nc.scalar.activation(a[:], xs[ib][:, c0:c1], ACTF.Abs,
                                     bias=qb_s[:, q:q + 1], scale=CBRT4)
                mt = mpool.tile([128, w], F16, tag=f"m{tag}", bufs=bufs)
                nc.vector.tensor_scalar(out=mt[:], in0=a[:],
                                        scalar1=-CBRT4, scalar2=0.0,
                                        op0=AOT.add, op1=AOT.min)
                p = ppool.tile([128, w], F16, tag=f"p{tag}", bufs=bufs)
                nc.vector._custom_dve(B4PAIR, out=p[:], in0=a[:], in1=mt[:],
                                      s0=2.0 * CBRT4, s1=4.0)
                return p

            def plane_pair(ib, q0_, n):
                """n consecutive planes (n in 1..2) through ONE vector
                tensor_scalar + ONE custom op ([128, n, BC] tiles) to halve
                the per-op fixed overhead."""
                a = apool.tile([128, n, BC], F16, tag="apr", bufs=3)
                for j in range(n):
                    nc.scalar.activation(a[:, j, :], xs[ib][:], ACTF.Abs,
                                         bias=qb_s[:, q0_ + j:q0_ + j + 1],
                                         scale=CBRT4)
                mt = mpool.tile([128, n, BC], F16, tag="mpr", bufs=3)
                nc.vector.tensor_scalar(out=mt[:], in0=a[:],
                                        scalar1=-CBRT4, scalar2=0.0,
                                        op0=AOT.add, op1=AOT.min)
                p = ppool.tile([128, n, BC], F16, tag="ppr", bufs=4)
                nc.vector._custom_dve(B4PAIR, out=p[:], in0=a[:], in1=mt[:],
                                      s0=2.0 * CBRT4, s1=4.0)
                return p

            def mm(p, pc0, kt, bb, start, stop):
                lo = bb * 128 - pc0
                nc.tensor.matmul(psums[bb][:], p[:, lo:lo + 128],
                                 h2_s[:, kt, :], start=start, stop=stop)

            # --- i-block 0: q0/q1 sliced to DMA arrival; banks 4-7 deferred.
            pA = plane_ops(0, 0, 0, 256, "s4", 2)       # q0 cols 0:256
            mm(pA, 0, 0, 0, True, False)
            mm(pA, 0, 0, 1, True, False)
            pB = plane_ops(0, 0, 256, 512, "s4", 2)     # q0 cols 256:512
            mm(pB, 256, 0, 2, True, False)
            mm(pB, 256, 0, 3, True, False)
            pC = plane_ops(0, 1, 0, 512, "s2", 2)       # q1 cols 0:512
            for bb in range(4):
                mm(pC, 0, 1, bb, False, False)
            pD = plane_ops(0, 0, 512, 1024, "s2", 2)    # q0 cols 512:1024
            for bb in range(4, 8):
                mm(pD, 512, 0, bb, True, False)
            pE = plane_ops(0, 1, 512, 1024, "s2", 2)    # q1 cols 512:1024
            for bb in range(4, 8):
                mm(pE, 512, 1, bb, False, False)
            for q in (2, 3, 4):
                p = plane_ops(0, q, 0, BC, "", None)    # singles keep latency
                for bb in range(NBB):                   # low during ramp-up
                    mm(p, 0, q, bb, False, False)
            p56 = plane_pair(0, 5, 2)
            for j in range(2):
                for bb in range(NBB):
                    mm(p56[:, j, :], 0, 5 + j, bb, False, False)

            # mid-stream input DMAs on the scalar queue (slack after ib0).
            nc.scalar.dma_start(out=xs[1][:], in_=xt_d[128:256, :])
            nc.scalar.dma_start(out=xs[3][:], in_=xt_d[384:512, :])
            h2_dma(nc.scalar, 12, 15)

            # --- steady stream: paired planes, kt-major; ib3's planes kept
            # for the bank-major finale (production runs ~0.5us/plane ahead,
            # so the sweep is pure PE speed).
            planes = {}
            for ib in range(1, IB):
                for q0_ in (0, 2, 4, 6):
                    n = 1 if q0_ == 6 else 2
                    p = plane_pair(ib, q0_, n)
                    for j in range(n):
                        q = q0_ + j
                        if ib < IB - 1:
                            for bb in range(NBB):
                                mm(p[:, j, :], 0, ib * Q + q, bb,
                                   False, False)
                        else:
                            planes[q] = p[:, j, :]

            # last i-block: bank-major so each PSUM bank finishes ~1.5us
            # apart and its drain + out-DMA overlaps the remaining matmuls.
            for bb in range(NBB):
                for q in range(Q):
                    kt = (IB - 1) * Q + q
                    lhs = planes[q][:, bb * 128:(bb + 1) * 128]
                    if bb == NBB - 1 and q == Q - 1:
                        nc.tensor.matmul(psums[bb][:, 0:256], lhs,
                                         h2_s[:, kt, 0:256],
                                         start=False, stop=True)
                        nc.tensor.matmul(psums[bb][:, 256:O], lhs,
                                         h2_s[:, kt, 256:O],
                                         start=False, stop=True)
                        continue
                    nc.tensor.matmul(psums[bb][:], lhs, h2_s[:, kt, :],
                                     start=False, stop=(q == Q - 1))
                if bb in (1, 3, 5):
                    j = bb // 2
                    o2 = opool.tile([128, 2, O], F32, tag="o", bufs=2)
                    nc.scalar.copy(o2[:, 0, :], psums[2 * j][:])
                    nc.vector.tensor_copy(out=o2[:, 1, :],
                                          in_=psums[2 * j + 1][:])
                    nc.sync.dma_start(
                        out=out_d[2 * j * 128:(2 * j + 2) * 128, :].rearrange(
                            "(k p) o -> p k o", p=128),
                        in_=o2[:])
                elif bb == 6:
                    o6 = opool.tile([128, O], F32, tag="o1", bufs=1)
                    nc.scalar.copy(o6[:, 0:256], psums[6][:, 0:256])
                    nc.scalar.dma_start(out=out_d[6 * 128:7 * 128, 0:256],
                                        in_=o6[:, 0:256])
                    nc.vector.tensor_copy(out=o6[:, 256:O],
                                          in_=psums[6][:, 256:O])
                    nc.sync.dma_start(out=out_d[6 * 128:7 * 128, 256:O],
                                      in_=o6[:, 256:O])
                elif bb == 7:
                    # last bank: quartered drain, out-DMA over three queues.
                    o7 = opool.tile([128, O], F32, tag="o2", bufs=1)
                    nc.vector.tensor_copy(out=o7[:, 0:128],
                                          in_=psums[7][:, 0:128])
                    nc.sync.dma_start(out=out_d[7 * 128:8 * 128, 0:128],
                                      in_=o7[:, 0:128])
                    nc.scalar.copy(o7[:, 128:256], psums[7][:, 128:256])
                    nc.scalar.dma_start(out=out_d[7 * 128:8 * 128, 128:256],
                                        in_=o7[:, 128:256])
                    nc.vector.tensor_copy(out=o7[:, 256:384],
                                          in_=psums[7][:, 256:384])
                    nc.gpsimd.dma_start(out=out_d[7 * 128:8 * 128, 256:384],
                                        in_=o7[:, 256:384])
                    nc.scalar.copy(o7[:, 384:O], psums[7][:, 384:O])
                    nc.sync.dma_start(out=out_d[7 * 128:8 * 128, 384:O],
                                      in_=o7[:, 384:O])

    nc.compile()
    return nc


def _get_program():
    if "p" not in _programs:
        _programs["p"] = _build_program()
    return _programs["p"]


def _host_prep(x, weights, coefficients):
    x = np.asarray(x, dtype=np.float32)
    weights = np.asarray(weights, dtype=np.float32)
    coefficients = np.asarray(coefficients, dtype=np.float32)

    # raw truncated-power coefficients G_q = sum_g w5[q-g]/6 * C2_g
    c2 = coefficients.astype(np.float64) * weights.astype(np.float64)[:, :, None]
    c2 = c2.transpose(2, 1, 0)                     # [G, I, O]
    w5 = np.array([1.0, -4.0, 6.0, -4.0, 1.0]) / 6.0
    graw = np.zeros((Q, I, O), dtype=np.float64)
    for q in range(Q):
        for g in range(G):
            r = q - g
            if 0 <= r <= 4:
                graw[q] += w5[r] * c2[g]
    # local planes P_q = 4 * [V_q -4V_{q+1} +6V_{q+2} -4V_{q+3} +V_{q+4}]
    # (indices >= 7 vanish on the domain) => H = (M4^T)^{-1} G / 4
    w4 = np.array([1.0, -4.0, 6.0, -4.0, 1.0])
    M4 = np.eye(Q)
    for q in range(Q):
        for r in range(1, 5):
            if q + r < Q:
                M4[q, q + r] = w4[r]
    h = np.linalg.solve(M4.T, graw.reshape(Q, -1)).reshape(Q, I, O) / 4.0
    # device row order kt = ib*7 + q
    h2k = np.empty((KT, 128, O), dtype=np.float16)
    for ib in range(IB):
        for q in range(Q):
            h2k[ib * Q + q] = h[q, ib * 128:(ib + 1) * 128, :].astype(
                np.float16)
    h2k = np.ascontiguousarray(h2k.reshape(KT * 128, O))

    # tp = min(3.5*x, 3.5); planes vanish for t<0 via the |.| window.
    tp = np.minimum(3.5 * x, np.float32(3.5)).astype(np.float16)
    xt = np.ascontiguousarray(tp.T)                # [I, B] fp16
    # s'_q = |c*tp + c*(1.5 - q)| = c*|t - (q+2)|,  t = tp + 3.5
    qb = np.tile((CBRT4 * (1.5 - np.arange(8, dtype=np.float64))
                  ).astype(np.float32)[None, :], (128, 1))

    in_maps = []
    for c in range(N_CORES):
        in_maps.append({
            "xt": np.ascontiguousarray(xt[:, c * BC:(c + 1) * BC]),
            "h2": h2k,
            "qb": qb,
        })
    return in_maps


def _run(x, weights, coefficients, **spmd_kwargs):
    nc = _get_program()
    in_maps = _host_prep(x, weights, coefficients)
    res = run_bass_kernel_spmd(nc, in_maps, list(range(N_CORES)), **spmd_kwargs)
    out = np.concatenate([res.results[c]["out"] for c in range(N_CORES)], axis=0)
    return out.astype(np.float32), res


def kernel(x, weights, coefficients):
    out, _ = _run(x, weights, coefficients)
    return out
